# revision 18
# baseline (speedup 1.0000x reference)
"""Trainium2 Bass kernel for nn_MoE_90297392431448.

MoE layer: B=2, T=2048, D=1024, H=4096, E=8 experts, top-K=2 routing.

Strategy (expert-parallel, tokens always the streaming free dim):
  - Host: gating softmax + top-2 selection in fp64, renormalized gate
    weights; gather each expert's tokens.
  - The per-core column space is NSLOT uniform slots whose sizes come
    from a small packing solver: each slot holds one expert's tokens and
    its own streamed weight set, and an expert's token count is covered
    by a multiset of slots across cores. 3 solver-chosen slot sizes cut
    per-core padding from M~1058 (pair scheme) to ~1030 columns vs the
    perfect balance of 1024, at the cost of streaming a third weight set
    per core. All 8 cores run ONE uniform program (single NEFF, SPMD);
    if the solver finds nothing good it falls back to the 2-slot
    big-with-small expert pairing.
  - Device, two-stage FFN with TOKENS as the moving/free dimension in
    BOTH stages (cost proportional to M, no 128-token quantization):
        stage 1: ht[h, m] = gelu(sum_k W1[k, h] * xT[k, m] + b1[h])
        stage 2: y[d, m]  = sum_h W2[h, d] * ht[h, m]
    bf16 matmuls, fp32 PSUM accumulation, fp16 output store. Stage 1
    runs k-OUTER per c-chunk (one PSUM bank per column block, all blocks
    accumulating concurrently) so the first real matmul needs only the
    first xt k-chunk + first W1 slices instead of the whole fill.
  - W1 streams through an SBUF ring in consumption order, interleaved
    with xt across the sync+scalar queues at the start; W2 streams
    mostly during stage 2 where DMA is otherwise idle.
  - Host: scale columns by gate weights, scatter-add into [B,T,D], plus
    the w*b2 rank-1 term.
"""

import itertools
import numpy as np
import ml_dtypes
from functools import lru_cache

B, T, D, H, E, K = 2, 2048, 1024, 4096, 8, 2
P = 128
KD = D // P
CH = H // P
DT = D // P
N_TOK = B * T
N_WARM = 20

_compiled_cache = {}
_solve_cache = {}


def _split_cap(cap):
    if cap <= 0:
        return []
    nb = -(-cap // 512)
    base = cap // nb
    rem = cap - base * nb
    return [base + (1 if i < rem else 0) for i in range(nb)]


def _blocks(caps):
    blocks = []
    col = 0
    for slot, cap in enumerate(caps):
        for n in _split_cap(cap):
            blocks.append((col, n, slot))
            col += n
    return blocks


def _combos(sizes):
    out = []
    for js in itertools.product(range(4), repeat=len(sizes)):
        n = sum(js)
        if n == 0 or n > 3:
            continue
        out.append((sum(j * s for j, s in zip(js, sizes)), js))
    out.sort()
    return out


def _assign(sizes, counts, want_assignment=False):
    """Can the sorted-desc counts be covered by 8 copies of each slot size
    (each expert using <=3 slots, minimal covers)? Optionally return the
    per-expert slot multiset."""
    combos = _combos(sizes)
    smin = min(sizes)
    res = []

    @lru_cache(maxsize=None)
    def rec(i, avail):
        if i == len(counts):
            return True
        for cov, js in combos:
            if cov < counts[i]:
                continue
            if cov - counts[i] >= smin:
                continue
            if all(a >= j for a, j in zip(avail, js)):
                if rec(i + 1, tuple(a - j for a, j in zip(avail, js))):
                    if want_assignment:
                        res.append(js)
                    return True
        return False

    ok = rec(0, (8,) * len(sizes))
    if not ok:
        return None
    if want_assignment:
        return list(reversed(res))
    return True


def _solve_slots(counts, m2):
    """Search 3-slot sizes minimizing M = s1+s2+s3; only totals that beat
    the 2-slot M by >=12 columns are worth the extra weight stream."""
    key = tuple(counts)
    if key in _solve_cache:
        return _solve_cache[key]
    import time
    t0 = time.time()
    cs = tuple(sorted(counts, reverse=True))
    best = None
    for tot in range(max(1024, 3 * 64), min(m2 - 12, 1072)):
        if time.time() - t0 > 45:
            break
        found = None
        for s1 in range(max((tot + 2) // 3, (cs[0] + 2) // 3), min(512, tot - 128) + 1):
            for s2 in range((tot - s1 + 1) // 2, s1 + 1):
                s3 = tot - s1 - s2
                if s3 < 64 or s3 > s2:
                    continue
                if _assign((s1, s2, s3), cs):
                    found = (s1, s2, s3)
                    break
            if found:
                break
        if found:
            best = (tot, found, _assign(found, cs, want_assignment=True))
            break
    _solve_cache[key] = best
    return best


def _build(caps):
    """Build + compile the uniform per-core kernel for the slot caps."""
    import concourse.mybir as mybir
    import concourse.tile as tile
    from concourse import bacc

    bf16 = mybir.dt.bfloat16
    f32 = mybir.dt.float32
    f16 = mybir.dt.float16

    NS = len(caps)
    M = sum(caps)
    blocks = _blocks(caps)
    NB = len(blocks)
    W1C = KD * NS * P
    W2C = NS * CH * P

    nc = bacc.Bacc("TRN2", target_bir_lowering=False, debug=False, num_devices=E)

    xt_d = nc.dram_tensor("xt", [P, KD * M], bf16, kind="ExternalInput")
    w1_d = nc.dram_tensor("w1", [P, CH * W1C], bf16, kind="ExternalInput")
    w2_d = nc.dram_tensor("w2", [P, DT * W2C], bf16, kind="ExternalInput")
    b1_d = nc.dram_tensor("b1s", [P, CH * NS], f32, kind="ExternalInput")
    y_d = nc.dram_tensor("y", [D, M], f16, kind="ExternalOutput")

    w1bufs = 10 if NS == 2 else 8

    with tile.TileContext(nc) as tc:
        with (
            tc.tile_pool(name="xin", bufs=1) as xpool,
            tc.tile_pool(name="w1r", bufs=w1bufs) as w1pool,
            tc.tile_pool(name="w2r", bufs=2) as w2pool,
            tc.tile_pool(name="hbuf", bufs=1) as hpool,
            tc.tile_pool(name="obuf", bufs=4) as opool,
            tc.tile_pool(name="ps", bufs=min(8, 2 * NB), space="PSUM") as pspool,
        ):
            wz = xpool.tile([P, 512], bf16, tag="warmsrc")
            nc.vector.memset(wz[:], 0.0)
            pw = pspool.tile([P, 512], f32, tag="ps", name="pw")
            for _ in range(N_WARM):
                nc.tensor.matmul(pw[:], wz[:, :P], wz[:], start=True, stop=True)
            wg = xpool.tile([P, 1], bf16, tag="warmgelu")
            nc.scalar.activation(wg[:], wz[:, :1],
                                 mybir.ActivationFunctionType.Gelu)

            xt = xpool.tile([P, KD * M], bf16, tag="xt")
            b1 = xpool.tile([P, CH * NS], f32, tag="b1")
            ht = hpool.tile([P, CH, M], bf16, tag="ht")

            w1tiles = {}

            def w1_alloc(c):
                w1tiles[c] = w1pool.tile([P, W1C], bf16, tag="w1c",
                                         name=f"w1c{c}")
                return w1tiles[c]

            def w1_dma(c, eng, lo=0, hi=W1C):
                eng.dma_start(w1tiles[c][:, lo:hi],
                              w1_d.ap()[:, c * W1C + lo: c * W1C + hi])

            def xt_dma(k, eng, lo=0, hi=M):
                eng.dma_start(xt[:, k * M + lo: k * M + hi],
                              xt_d.ap()[:, k * M + lo: k * M + hi])

            # Early stream in consumption order across sync+scalar (the DGE
            # shares bandwidth between active queues; gpsimd is slow — it
            # only carries W2 plus, for NS=3, the odd half of the late W1
            # stream, which one queue alone cannot sustain; scalar must stay
            # clear of ring-gated DMAs or it would deadlock behind its own
            # gelus).
            nc.scalar.dma_start(b1[:], b1_d.ap())
            w1_alloc(0)
            w1_dma(0, nc.sync, 0, 2 * NS * P)
            xt_dma(0, nc.sync)
            xt_dma(1, nc.scalar)
            w1_dma(0, nc.scalar, 2 * NS * P, W1C)
            xt_dma(2, nc.sync)
            xt_dma(3, nc.scalar)
            w1_alloc(1)
            w1_dma(1, nc.sync)
            xt_dma(4, nc.sync)
            xt_dma(5, nc.scalar)
            xt_dma(6, nc.sync)
            xt_dma(7, nc.scalar)
            w1_alloc(2)
            w1_dma(2, nc.scalar)
            w1_alloc(3)
            w1_dma(3, nc.sync)
            for c in range(4, CH):
                w1_alloc(c)
                if NS >= 3 and c >= 8 and c % 2 == 1:
                    w1_dma(c, nc.gpsimd)
                else:
                    w1_dma(c, nc.sync)

            acts = []
            for c in range(CH):
                w1t = w1tiles[c]
                pss = [pspool.tile([P, 512], f32, tag="ps", name="ps1")
                       for _ in blocks]
                for k in range(KD):
                    for bi, (col0, n, slot) in enumerate(blocks):
                        nc.tensor.matmul(
                            pss[bi][:, :n],
                            w1t[:, k * NS * P + slot * P:
                                k * NS * P + (slot + 1) * P],
                            xt[:, k * M + col0: k * M + col0 + n],
                            start=(k == 0),
                            stop=(k == KD - 1),
                        )
                for bi, (col0, n, slot) in enumerate(blocks):
                    act = nc.scalar.activation(
                        ht[:, c, col0:col0 + n], pss[bi][:, :n],
                        mybir.ActivationFunctionType.Gelu,
                        bias=b1[:, c * NS + slot: c * NS + slot + 1],
                    )
                    if bi == 0:
                        acts.append(act)

            for d in range(DT):
                w2t = w2pool.tile([P, W2C], bf16, tag="w2c")
                if NS >= 3:
                    # W2 consumption in stage 2 is ~220GB/s for NS=3 — more
                    # than one queue sustains. d0/d1 ride sync (behind the
                    # tail of its W1 stream), the rest alternate gpsimd and
                    # scalar (both otherwise idle in stage 2).
                    w2eng = (nc.sync, nc.sync, nc.gpsimd, nc.scalar,
                             nc.gpsimd, nc.scalar, nc.gpsimd, nc.scalar)[d]
                    gate_c = (16, 20)[d] if d < 2 else None
                else:
                    w2eng = nc.gpsimd
                    gate_c = (22, 27)[d] if d < 2 else None
                dma = w2eng.dma_start(w2t[:],
                                      w2_d.ap()[:, d * W2C:(d + 1) * W2C])
                if gate_c is not None:
                    gate = acts[min(gate_c, CH - 2)]
                    tile.add_dep_helper(dma.ins, gate.ins,
                                        reason="pace W2 prefetch behind stage-1")
                for bi, (col0, n, slot) in enumerate(blocks):
                    ps2 = pspool.tile([P, 512], f32, tag="ps", name="ps2")
                    for h in range(CH):
                        nc.tensor.matmul(
                            ps2[:, :n],
                            w2t[:, slot * CH * P + h * P:
                                slot * CH * P + (h + 1) * P],
                            ht[:, h, col0:col0 + n],
                            start=(h == 0),
                            stop=(h == CH - 1),
                        )
                    ot = opool.tile([P, 512], f16, tag="ot")
                    nc.vector.tensor_copy(ot[:, :n], ps2[:, :n])
                    if d == DT - 1:
                        eng = (nc.sync, nc.scalar, nc.gpsimd)[bi % 3]
                    else:
                        eng = nc.sync if d % 2 == 0 else nc.scalar
                    eng.dma_start(y_d.ap()[d * P:(d + 1) * P,
                                           col0:col0 + n], ot[:, :n])
    nc.compile()
    return nc


def _route(x2d, Wg, bg):
    logits = x2d.astype(np.float64) @ Wg.astype(np.float64) + bg.astype(np.float64)
    m = logits.max(-1, keepdims=True)
    e = np.exp(logits - m)
    gates = e / e.sum(-1, keepdims=True)
    top2 = np.argsort(-gates, axis=-1, kind="stable")[:, :K]
    g2 = np.take_along_axis(gates, top2, axis=-1)
    w2 = g2 / np.maximum(g2.sum(-1, keepdims=True), 1e-12)
    return top2, w2


def _pack_w1(W1e, bf):
    # [D, H] -> [P(d'), CH, KD, P(h')]
    return W1e.astype(bf).reshape(KD, P, CH, P).transpose(1, 2, 0, 3)


def _pack_w2(W2e, bf):
    # [H, D] -> [P(h'), DT, CH, P(d')]
    return W2e.astype(bf).reshape(CH, P, DT, P).transpose(1, 2, 0, 3)


def _plan(counts):
    """Decide slot caps + per-(core,slot) expert/token assignment.

    Returns (caps, slot_expert[core][slot], slot_tokens[core][slot]) where
    slot_tokens are (start,stop) ranges into the expert's token list."""
    E_ = len(counts)
    order = np.argsort(-np.asarray(counts), kind="stable")
    bigs, smalls = order[:4], order[4:]
    capL = max(64, -(-int(counts[bigs[0]]) // 2))
    capS = max(64, -(-int(counts[smalls[0]]) // 2))
    m2 = capS + capL

    sol = _solve_slots(tuple(int(c) for c in counts), m2)
    if sol is not None:
        tot, sizes, assign = sol
        cs_order = np.argsort(-np.asarray(counts), kind="stable")
        NS = len(sizes)
        # expand per size class: list of experts using that class
        inst = [[] for _ in range(NS)]
        for rank, e in enumerate(cs_order):
            js = assign[rank]
            for q in range(NS):
                inst[q] += [int(e)] * js[q]
        for q in range(NS):
            inst[q] += [-1] * (8 - len(inst[q]))
        # split each expert's tokens across its slots in class-major order
        offs = {int(e): 0 for e in range(E_)}
        slot_expert = [[-1] * NS for _ in range(8)]
        slot_range = [[(0, 0)] * NS for _ in range(8)]
        for q in range(NS):
            for core in range(8):
                e = inst[q][core]
                slot_expert[core][q] = e
                if e < 0:
                    continue
                a = offs[e]
                b = min(a + sizes[q], int(counts[e]))
                offs[e] = b
                slot_range[core][q] = (a, b)
        assert all(offs[e] >= counts[e] for e in range(E_) if counts[e] > 0), \
            (offs, counts)
        return tuple(sizes), slot_expert, slot_range
    # 2-slot pair fallback
    caps = (capS, capL)
    slot_expert = []
    slot_range = []
    for i in range(4):
        eA, eB = int(bigs[i]), int(smalls[i])
        hA = -(-int(counts[eA]) // 2)
        hB = -(-int(counts[eB]) // 2)
        for half in range(2):
            slot_expert.append([eB, eA])
            slot_range.append([
                (0, hB) if half == 0 else (hB, int(counts[eB])),
                (0, hA) if half == 0 else (hA, int(counts[eA])),
            ])
    return caps, slot_expert, slot_range


def kernel(x, Wg, bg, W1, b1, W2, b2, _run_opts=None):
    from concourse.bass_utils import run_bass_kernel_spmd

    x = np.asarray(x)
    x2d = x.reshape(N_TOK, D)
    top2, wgt2 = _route(x2d, np.asarray(Wg), np.asarray(bg))

    pos = [np.where((top2 == e).any(-1))[0] for e in range(E)]
    pw = [
        (wgt2 * (top2 == e))[pos[e]].sum(-1).astype(np.float32)
        for e in range(E)
    ]
    counts = np.array([len(p) for p in pos])

    caps, slot_expert, slot_range = _plan(counts)
    NS = len(caps)
    M = sum(caps)
    col0s = np.concatenate([[0], np.cumsum(caps)])

    if caps not in _compiled_cache:
        _compiled_cache[caps] = _build(caps)
    nc = _compiled_cache[caps]

    bf = ml_dtypes.bfloat16
    W1 = np.asarray(W1)
    W2 = np.asarray(W2)
    b1 = np.asarray(b1)
    b2 = np.asarray(b2)

    pW1 = [_pack_w1(W1[e], bf) for e in range(E)]
    pW2 = [_pack_w2(W2[e], bf) for e in range(E)]

    in_maps = []
    core_slots = []  # per core: [(expert, rows, weights, col0), ...]
    for core in range(8):
        w1img = np.empty((P, CH, KD, NS, P), bf)
        w2img = np.empty((P, DT, NS, CH, P), bf)
        b1img = np.empty((P, CH, NS), np.float32)
        xtc = np.zeros((D, M), bf)
        cslots = []
        for q in range(NS):
            e = slot_expert[core][q]
            esrc = max(e, 0)
            w1img[:, :, :, q, :] = pW1[esrc]
            w2img[:, :, q, :, :] = pW2[esrc]
            b1img[:, :, q] = b1[esrc].reshape(CH, P).T
            if e < 0:
                continue
            a, bnd = slot_range[core][q]
            rows = pos[e][a:bnd]
            w = pw[e][a:bnd]
            xtc[:, col0s[q]:col0s[q] + len(rows)] = x2d[rows].T
            cslots.append((e, rows, w, int(col0s[q])))
        xtp = np.ascontiguousarray(
            xtc.reshape(KD, P, M).transpose(1, 0, 2).reshape(P, KD * M))
        in_maps.append({
            "xt": xtp,
            "w1": np.ascontiguousarray(w1img.reshape(P, CH * KD * NS * P)),
            "w2": np.ascontiguousarray(w2img.reshape(P, DT * NS * CH * P)),
            "b1s": np.ascontiguousarray(b1img.reshape(P, CH * NS)),
        })
        core_slots.append(cslots)

    try:
        res = run_bass_kernel_spmd(nc, in_maps, core_ids=list(range(E)),
                                   **(_run_opts or {}))
    except Exception:
        res = run_bass_kernel_spmd(nc, in_maps, core_ids=list(range(E)),
                                   **(_run_opts or {}))

    out = np.zeros((N_TOK, D), np.float32)
    for core in range(E):
        y = np.asarray(res.results[core]["y"]).astype(np.float32)  # [D, M]
        for e, rows, w, col0 in core_slots[core]:
            n = len(rows)
            if n == 0:
                continue
            out[rows] += (w[:, None] * y[:, col0:col0 + n].T
                          + w[:, None] * b2[e][None, :].astype(np.float32))
    if _run_opts is not None:
        kernel._last_result = res
    return out.reshape(B, T, D)


# revision 19
# speedup vs baseline: 1.1758x; 1.1758x over previous
"""Trainium2 Bass kernel for nn_MoE_90297392431448.

MoE layer: B=2, T=2048, D=1024, H=4096, E=8 experts, top-K=2 routing.

Strategy (expert-parallel, tokens always the streaming free dim):
  - Host: gating softmax + top-2 selection in fp64, renormalized gate
    weights; gather each expert's tokens.
  - The per-core column space is NSLOT uniform slots whose sizes come
    from a small packing solver: each slot holds one expert's tokens and
    its own streamed weight set, and an expert's token count is covered
    by a multiset of slots across cores. 3 solver-chosen slot sizes cut
    per-core padding from M~1058 (pair scheme) to ~1030 columns vs the
    perfect balance of 1024, at the cost of streaming a third weight set
    per core. All 8 cores run ONE uniform program (single NEFF, SPMD);
    if the solver finds nothing good it falls back to the 2-slot
    big-with-small expert pairing.
  - Device, two-stage FFN with TOKENS as the moving/free dimension in
    BOTH stages (cost proportional to M, no 128-token quantization):
        stage 1: ht[h, m] = gelu(sum_k W1[k, h] * xT[k, m] + b1[h])
        stage 2: y[d, m]  = sum_h W2[h, d] * ht[h, m]
    bf16 matmuls, fp32 PSUM accumulation, fp16 output store. Stage 1
    runs k-OUTER per c-chunk (one PSUM bank per column block, all blocks
    accumulating concurrently) so the first real matmul needs only the
    first xt k-chunk + first W1 slices instead of the whole fill.
  - W1 streams through an SBUF ring in consumption order, interleaved
    with xt across the sync+scalar queues at the start; W2 streams
    mostly during stage 2 where DMA is otherwise idle.
  - Host: scale columns by gate weights, scatter-add into [B,T,D], plus
    the w*b2 rank-1 term.
"""

import itertools
import numpy as np
import ml_dtypes
from functools import lru_cache

B, T, D, H, E, K = 2, 2048, 1024, 4096, 8, 2
P = 128
KD = D // P
CH = H // P
DT = D // P
N_TOK = B * T
N_WARM = 6

_compiled_cache = {}
_solve_cache = {}


def _split_cap(cap):
    if cap <= 0:
        return []
    nb = -(-cap // 512)
    base = cap // nb
    rem = cap - base * nb
    return [base + (1 if i < rem else 0) for i in range(nb)]


def _blocks(caps):
    blocks = []
    col = 0
    for slot, cap in enumerate(caps):
        for n in _split_cap(cap):
            blocks.append((col, n, slot))
            col += n
    return blocks


def _combos(sizes):
    out = []
    for js in itertools.product(range(4), repeat=len(sizes)):
        n = sum(js)
        if n == 0 or n > 3:
            continue
        out.append((sum(j * s for j, s in zip(js, sizes)), js))
    out.sort()
    return out


def _assign(sizes, counts, want_assignment=False):
    """Can the sorted-desc counts be covered by 8 copies of each slot size
    (each expert using <=3 slots, minimal covers)? Optionally return the
    per-expert slot multiset."""
    combos = _combos(sizes)
    smin = min(sizes)
    res = []

    @lru_cache(maxsize=None)
    def rec(i, avail):
        if i == len(counts):
            return True
        for cov, js in combos:
            if cov < counts[i]:
                continue
            if cov - counts[i] >= smin:
                continue
            if all(a >= j for a, j in zip(avail, js)):
                if rec(i + 1, tuple(a - j for a, j in zip(avail, js))):
                    if want_assignment:
                        res.append(js)
                    return True
        return False

    ok = rec(0, (8,) * len(sizes))
    if not ok:
        return None
    if want_assignment:
        return list(reversed(res))
    return True


def _solve_slots(counts, m2):
    """Search 3-slot sizes minimizing M = s1+s2+s3; only totals that beat
    the 2-slot M by >=12 columns are worth the extra weight stream."""
    key = tuple(counts)
    if key in _solve_cache:
        return _solve_cache[key]
    import time
    t0 = time.time()
    cs = tuple(sorted(counts, reverse=True))
    best = None
    for tot in range(max(1024, 3 * 64), min(m2 - 12, 1072)):
        if time.time() - t0 > 45:
            break
        found = None
        for s1 in range(max((tot + 2) // 3, (cs[0] + 2) // 3), min(512, tot - 128) + 1):
            for s2 in range((tot - s1 + 1) // 2, s1 + 1):
                s3 = tot - s1 - s2
                if s3 < 64 or s3 > s2:
                    continue
                if _assign((s1, s2, s3), cs):
                    found = (s1, s2, s3)
                    break
            if found:
                break
        if found:
            best = (tot, found, _assign(found, cs, want_assignment=True))
            break
    _solve_cache[key] = best
    return best


def _build(caps):
    """Build + compile the uniform per-core kernel for the slot caps."""
    import concourse.mybir as mybir
    import concourse.tile as tile
    from concourse import bacc

    bf16 = mybir.dt.bfloat16
    f32 = mybir.dt.float32
    f16 = mybir.dt.float16

    NS = len(caps)
    M = sum(caps)
    blocks = _blocks(caps)
    NB = len(blocks)
    W1C = KD * NS * P
    W2C = NS * CH * P

    nc = bacc.Bacc("TRN2", target_bir_lowering=False, debug=False, num_devices=E)

    xt_d = nc.dram_tensor("xt", [P, KD * M], bf16, kind="ExternalInput")
    w1_d = nc.dram_tensor("w1", [P, CH * W1C], bf16, kind="ExternalInput")
    w2_d = nc.dram_tensor("w2", [P, DT * W2C], bf16, kind="ExternalInput")
    b1_d = nc.dram_tensor("b1s", [P, CH * NS], f32, kind="ExternalInput")
    y_d = nc.dram_tensor("y", [D, M], f16, kind="ExternalOutput")

    w1bufs = 10 if NS == 2 else 8

    with tile.TileContext(nc) as tc:
        with (
            tc.tile_pool(name="xin", bufs=1) as xpool,
            tc.tile_pool(name="w1r", bufs=w1bufs) as w1pool,
            tc.tile_pool(name="w2r", bufs=2) as w2pool,
            tc.tile_pool(name="hbuf", bufs=1) as hpool,
            tc.tile_pool(name="obuf", bufs=4) as opool,
            tc.tile_pool(name="ps", bufs=min(8, 2 * NB), space="PSUM") as pspool,
        ):
            wz = xpool.tile([P, 512], bf16, tag="warmsrc")
            nc.vector.memset(wz[:], 0.0)
            pw = pspool.tile([P, 512], f32, tag="ps", name="pw")
            for _ in range(N_WARM):
                nc.tensor.matmul(pw[:], wz[:, :P], wz[:], start=True, stop=True)
            wg = xpool.tile([P, 1], bf16, tag="warmgelu")
            nc.scalar.activation(wg[:], wz[:, :1],
                                 mybir.ActivationFunctionType.Gelu)

            xt = xpool.tile([P, KD * M], bf16, tag="xt")
            b1 = xpool.tile([P, CH * NS], f32, tag="b1")
            ht = hpool.tile([P, CH, M], bf16, tag="ht")

            w1tiles = {}

            def w1_alloc(c):
                w1tiles[c] = w1pool.tile([P, W1C], bf16, tag="w1c",
                                         name=f"w1c{c}")
                return w1tiles[c]

            def w1_dma(c, eng, lo=0, hi=W1C):
                eng.dma_start(w1tiles[c][:, lo:hi],
                              w1_d.ap()[:, c * W1C + lo: c * W1C + hi])

            def xt_dma(k, eng, lo=0, hi=M):
                eng.dma_start(xt[:, k * M + lo: k * M + hi],
                              xt_d.ap()[:, k * M + lo: k * M + hi])

            # Early stream in consumption order across sync+scalar (the DGE
            # shares bandwidth between active queues; gpsimd is slow — it
            # only carries W2 plus, for NS=3, the odd half of the late W1
            # stream, which one queue alone cannot sustain; scalar must stay
            # clear of ring-gated DMAs or it would deadlock behind its own
            # gelus).
            nc.scalar.dma_start(b1[:], b1_d.ap())
            w1_alloc(0)
            w1_dma(0, nc.sync, 0, 2 * NS * P)
            xt_dma(0, nc.sync)
            xt_dma(1, nc.scalar)
            w1_dma(0, nc.scalar, 2 * NS * P, W1C)
            xt_dma(2, nc.sync)
            xt_dma(3, nc.scalar)
            w1_alloc(1)
            w1_dma(1, nc.sync)
            xt_dma(4, nc.sync)
            xt_dma(5, nc.scalar)
            xt_dma(6, nc.sync)
            xt_dma(7, nc.scalar)
            w1_alloc(2)
            w1_dma(2, nc.scalar)
            w1_alloc(3)
            w1_dma(3, nc.sync)
            for c in range(4, CH):
                w1_alloc(c)
                if NS >= 3 and c >= 8 and c % 2 == 1:
                    w1_dma(c, nc.gpsimd)
                else:
                    w1_dma(c, nc.sync)

            acts = []
            for c in range(CH):
                w1t = w1tiles[c]
                pss = [pspool.tile([P, 512], f32, tag="ps", name="ps1")
                       for _ in blocks]
                for k in range(KD):
                    for bi, (col0, n, slot) in enumerate(blocks):
                        nc.tensor.matmul(
                            pss[bi][:, :n],
                            w1t[:, k * NS * P + slot * P:
                                k * NS * P + (slot + 1) * P],
                            xt[:, k * M + col0: k * M + col0 + n],
                            start=(k == 0),
                            stop=(k == KD - 1),
                        )
                for bi, (col0, n, slot) in enumerate(blocks):
                    act = nc.scalar.activation(
                        ht[:, c, col0:col0 + n], pss[bi][:, :n],
                        mybir.ActivationFunctionType.Gelu,
                        bias=b1[:, c * NS + slot: c * NS + slot + 1],
                    )
                    if bi == 0:
                        acts.append(act)

            for d in range(DT):
                w2t = w2pool.tile([P, W2C], bf16, tag="w2c")
                if NS >= 3:
                    # W2 consumption in stage 2 is ~220GB/s for NS=3 — more
                    # than one queue sustains. d0/d1 ride sync (behind the
                    # tail of its W1 stream), the rest alternate gpsimd and
                    # scalar (both otherwise idle in stage 2).
                    w2eng = (nc.sync, nc.sync, nc.gpsimd, nc.scalar,
                             nc.gpsimd, nc.scalar, nc.gpsimd, nc.scalar)[d]
                    gate_c = (16, 20)[d] if d < 2 else None
                else:
                    w2eng = nc.gpsimd
                    gate_c = (22, 27)[d] if d < 2 else None
                dma = w2eng.dma_start(w2t[:],
                                      w2_d.ap()[:, d * W2C:(d + 1) * W2C])
                if gate_c is not None:
                    gate = acts[min(gate_c, CH - 2)]
                    tile.add_dep_helper(dma.ins, gate.ins,
                                        reason="pace W2 prefetch behind stage-1")
                for bi, (col0, n, slot) in enumerate(blocks):
                    ps2 = pspool.tile([P, 512], f32, tag="ps", name="ps2")
                    for h in range(CH):
                        nc.tensor.matmul(
                            ps2[:, :n],
                            w2t[:, slot * CH * P + h * P:
                                slot * CH * P + (h + 1) * P],
                            ht[:, h, col0:col0 + n],
                            start=(h == 0),
                            stop=(h == CH - 1),
                        )
                    ot = opool.tile([P, 512], f16, tag="ot")
                    nc.vector.tensor_copy(ot[:, :n], ps2[:, :n])
                    if d == DT - 1:
                        eng = (nc.sync, nc.scalar, nc.gpsimd)[bi % 3]
                    else:
                        eng = nc.sync if d % 2 == 0 else nc.scalar
                    eng.dma_start(y_d.ap()[d * P:(d + 1) * P,
                                           col0:col0 + n], ot[:, :n])
    nc.compile()
    return nc


def _route(x2d, Wg, bg):
    logits = x2d.astype(np.float64) @ Wg.astype(np.float64) + bg.astype(np.float64)
    m = logits.max(-1, keepdims=True)
    e = np.exp(logits - m)
    gates = e / e.sum(-1, keepdims=True)
    top2 = np.argsort(-gates, axis=-1, kind="stable")[:, :K]
    g2 = np.take_along_axis(gates, top2, axis=-1)
    w2 = g2 / np.maximum(g2.sum(-1, keepdims=True), 1e-12)
    return top2, w2


def _pack_w1(W1e, bf):
    # [D, H] -> [P(d'), CH, KD, P(h')]
    return W1e.astype(bf).reshape(KD, P, CH, P).transpose(1, 2, 0, 3)


def _pack_w2(W2e, bf):
    # [H, D] -> [P(h'), DT, CH, P(d')]
    return W2e.astype(bf).reshape(CH, P, DT, P).transpose(1, 2, 0, 3)


def _plan(counts):
    """Decide slot caps + per-(core,slot) expert/token assignment.

    Returns (caps, slot_expert[core][slot], slot_tokens[core][slot]) where
    slot_tokens are (start,stop) ranges into the expert's token list."""
    E_ = len(counts)
    order = np.argsort(-np.asarray(counts), kind="stable")
    bigs, smalls = order[:4], order[4:]
    capL = max(64, -(-int(counts[bigs[0]]) // 2))
    capS = max(64, -(-int(counts[smalls[0]]) // 2))
    m2 = capS + capL

    sol = _solve_slots(tuple(int(c) for c in counts), m2)
    if sol is not None:
        tot, sizes, assign = sol
        cs_order = np.argsort(-np.asarray(counts), kind="stable")
        NS = len(sizes)
        # expand per size class: list of experts using that class
        inst = [[] for _ in range(NS)]
        for rank, e in enumerate(cs_order):
            js = assign[rank]
            for q in range(NS):
                inst[q] += [int(e)] * js[q]
        for q in range(NS):
            inst[q] += [-1] * (8 - len(inst[q]))
        # split each expert's tokens across its slots in class-major order
        offs = {int(e): 0 for e in range(E_)}
        slot_expert = [[-1] * NS for _ in range(8)]
        slot_range = [[(0, 0)] * NS for _ in range(8)]
        for q in range(NS):
            for core in range(8):
                e = inst[q][core]
                slot_expert[core][q] = e
                if e < 0:
                    continue
                a = offs[e]
                b = min(a + sizes[q], int(counts[e]))
                offs[e] = b
                slot_range[core][q] = (a, b)
        assert all(offs[e] >= counts[e] for e in range(E_) if counts[e] > 0), \
            (offs, counts)
        return tuple(sizes), slot_expert, slot_range
    # 2-slot pair fallback
    caps = (capS, capL)
    slot_expert = []
    slot_range = []
    for i in range(4):
        eA, eB = int(bigs[i]), int(smalls[i])
        hA = -(-int(counts[eA]) // 2)
        hB = -(-int(counts[eB]) // 2)
        for half in range(2):
            slot_expert.append([eB, eA])
            slot_range.append([
                (0, hB) if half == 0 else (hB, int(counts[eB])),
                (0, hA) if half == 0 else (hA, int(counts[eA])),
            ])
    return caps, slot_expert, slot_range


def kernel(x, Wg, bg, W1, b1, W2, b2, _run_opts=None):
    from concourse.bass_utils import run_bass_kernel_spmd

    x = np.asarray(x)
    x2d = x.reshape(N_TOK, D)
    top2, wgt2 = _route(x2d, np.asarray(Wg), np.asarray(bg))

    pos = [np.where((top2 == e).any(-1))[0] for e in range(E)]
    pw = [
        (wgt2 * (top2 == e))[pos[e]].sum(-1).astype(np.float32)
        for e in range(E)
    ]
    counts = np.array([len(p) for p in pos])

    caps, slot_expert, slot_range = _plan(counts)
    NS = len(caps)
    M = sum(caps)
    col0s = np.concatenate([[0], np.cumsum(caps)])

    if caps not in _compiled_cache:
        _compiled_cache[caps] = _build(caps)
    nc = _compiled_cache[caps]

    bf = ml_dtypes.bfloat16
    W1 = np.asarray(W1)
    W2 = np.asarray(W2)
    b1 = np.asarray(b1)
    b2 = np.asarray(b2)

    pW1 = [_pack_w1(W1[e], bf) for e in range(E)]
    pW2 = [_pack_w2(W2[e], bf) for e in range(E)]

    in_maps = []
    core_slots = []  # per core: [(expert, rows, weights, col0), ...]
    for core in range(8):
        w1img = np.empty((P, CH, KD, NS, P), bf)
        w2img = np.empty((P, DT, NS, CH, P), bf)
        b1img = np.empty((P, CH, NS), np.float32)
        xtc = np.zeros((D, M), bf)
        cslots = []
        for q in range(NS):
            e = slot_expert[core][q]
            esrc = max(e, 0)
            w1img[:, :, :, q, :] = pW1[esrc]
            w2img[:, :, q, :, :] = pW2[esrc]
            b1img[:, :, q] = b1[esrc].reshape(CH, P).T
            if e < 0:
                continue
            a, bnd = slot_range[core][q]
            rows = pos[e][a:bnd]
            w = pw[e][a:bnd]
            xtc[:, col0s[q]:col0s[q] + len(rows)] = x2d[rows].T
            cslots.append((e, rows, w, int(col0s[q])))
        xtp = np.ascontiguousarray(
            xtc.reshape(KD, P, M).transpose(1, 0, 2).reshape(P, KD * M))
        in_maps.append({
            "xt": xtp,
            "w1": np.ascontiguousarray(w1img.reshape(P, CH * KD * NS * P)),
            "w2": np.ascontiguousarray(w2img.reshape(P, DT * NS * CH * P)),
            "b1s": np.ascontiguousarray(b1img.reshape(P, CH * NS)),
        })
        core_slots.append(cslots)

    try:
        res = run_bass_kernel_spmd(nc, in_maps, core_ids=list(range(E)),
                                   **(_run_opts or {}))
    except Exception:
        res = run_bass_kernel_spmd(nc, in_maps, core_ids=list(range(E)),
                                   **(_run_opts or {}))

    out = np.zeros((N_TOK, D), np.float32)
    for core in range(E):
        y = np.asarray(res.results[core]["y"]).astype(np.float32)  # [D, M]
        for e, rows, w, col0 in core_slots[core]:
            n = len(rows)
            if n == 0:
                continue
            out[rows] += (w[:, None] * y[:, col0:col0 + n].T
                          + w[:, None] * b2[e][None, :].astype(np.float32))
    if _run_opts is not None:
        kernel._last_result = res
    return out.reshape(B, T, D)
